# revision 9
# baseline (speedup 1.0000x reference)
"""Trainium2 Bass kernel for nn_Attention (B=2, N=2048, H=16, hd=64, D=1024).

Strategy (8 NeuronCores, no collectives):
  core c -> batch b=c//4, query chunk j=c%4 (512 rows). Each core computes
  K,V for its batch over the first KP=KT*128 key rows (KT specialized to the
  runtime vaild_num; masked tail keys contribute exp(-1e9)=0), Q for its own
  512 rows, attention in transposed layout (scores S^T[k,q] so the key-validity
  mask is a per-partition bias folded into the Exp activation), and the output
  projection. Per-sample valid-length semantics:
    - keys k >= v: masked via bias -1e9 before exp -> P=0
    - queries q >= v: reference gives uniform softmax over ALL 2048 keys ->
      out row = mean(V_full) @ W_proj + b_proj. Implemented by zeroing the
      normalizer for invalid q and adding a rank-1 fixup row in the proj
      matmul: out += (q>=v) * (mean(V) @ W_proj).
  Softmax denominators ride along the PV matmul as 16 extra stationary
  columns (diagonal ones), landing head h's denominator on PSUM partition
  64+h so all 16 can be batched for one reciprocal.

Compute dtype bf16 (fp32 PSUM accumulation); fp32 in/out.
"""

import numpy as np
import ml_dtypes

import concourse.mybir as mybir
import concourse.tile as tile
from concourse import bacc
from concourse.bass_utils import run_bass_kernel_spmd

F32 = mybir.dt.float32
BF16 = mybir.dt.bfloat16
AF = mybir.ActivationFunctionType
OP = mybir.AluOpType

H, HD, D, N, B, NCORES = 16, 64, 1024, 2048, 2, 8
QC = 512            # query rows per core
NEG = -1e9
BF = ml_dtypes.bfloat16


def build_nc(KT):
    """Build the SPMD graph for KT key tiles (KT*128 >= max valid length)."""
    KP = KT * 128
    # free-dim chunks for the K^T projection output (seq axis)
    kchunks = []
    off = 0
    while off < KP:
        w = min(512, KP - off)
        kchunks.append((off, w))
        off += w
    NST = KT          # number of 128-row seq tiles for V
    VW = 64 + 82 * 16  # vaug width: blocks stride 81, diag col at 82h+64

    nc = bacc.Bacc(None, target_bir_lowering=False)

    # ---------------- DRAM parameters (per-core host-prepared shards) -------
    xT_d = nc.declare_dram_parameter("xT", [D, N], BF16, isOutput=False)
    xTq_d = nc.declare_dram_parameter("xTq", [D, QC], BF16, isOutput=False)
    wqkv_d = nc.declare_dram_parameter("wqkv", [D, 3 * D], BF16, isOutput=False)
    wproj_d = nc.declare_dram_parameter("wproj", [D, D], BF16, isOutput=False)
    # per-partition biases: [128, 8] each; col ft covers features ft*128..+128
    # bqT pre-scaled by 1/sqrt(hd)
    biaspp_d = nc.declare_dram_parameter("biaspp", [128, 16], F32, isOutput=False)
    bvH_d = nc.declare_dram_parameter("bvH", [64, 16], F32, isOutput=False)  # V bias per head
    brows_d = nc.declare_dram_parameter("brows", [1, 2 * D], BF16, isOutput=False)  # [bv | bproj]
    qiota_d = nc.declare_dram_parameter("qiota", [128, QC], F32, isOutput=False)
    v128_d = nc.declare_dram_parameter("v128", [128, 1], F32, isOutput=False)
    kiota_d = nc.declare_dram_parameter("kiota", [128, KT], F32, isOutput=False)
    esel_d = nc.declare_dram_parameter("esel", [H, H * 64], BF16, isOutput=False)
    out_d = nc.declare_dram_parameter("out", [QC, D], F32, isOutput=True)

    with tile.TileContext(nc) as tc:
        with tc.tile_pool(name="const", bufs=1) as cpool, \
             tc.tile_pool(name="qkv", bufs=1) as qkvpool, \
             tc.tile_pool(name="wpp", bufs=1) as wppool, \
             tc.tile_pool(name="psA", bufs=3, space="PSUM") as psA, \
             tc.tile_pool(name="psPV", bufs=2, space="PSUM") as psPV, \
             tc.tile_pool(name="psR", bufs=2, space="PSUM") as psR, \
             tc.tile_pool(name="psS", bufs=1, space="PSUM") as psS:

            # ---------------- constants / small inputs ----------------
            biaspp = cpool.tile([128, 16], F32, tag="biaspp")
            bvH = cpool.tile([64, 16], F32, tag="bvH")
            v128 = cpool.tile([128, 1], F32, tag="v128")
            kiota = cpool.tile([128, KT], F32, tag="kiota")
            nc.sync.dma_start(out=biaspp[:, :], in_=biaspp_d[:, :])
            nc.sync.dma_start(out=bvH[:, :], in_=bvH_d[:, :])
            nc.sync.dma_start(out=v128[:, :], in_=v128_d[:, :])
            nc.sync.dma_start(out=kiota[:, :], in_=kiota_d[:, :])
            bvrow = cpool.tile([1, D], BF16, tag="bvrow")
            nc.sync.dma_start(out=bvrow[:, :], in_=brows_d[0:1, 0:D])
            ones1 = cpool.tile([1, 128], BF16, tag="ones1")
            nc.vector.memset(ones1[:, :], 1.0)
            kb = cpool.tile([128, KT], F32, tag="kb")
            nc.vector.tensor_scalar(out=kb[:, :], in0=kiota[:, :],
                                    scalar1=v128[:, 0:1], scalar2=NEG,
                                    op0=OP.is_ge, op1=OP.mult)
            meanVT = cpool.tile([64, H], BF16, tag="meanVT")
            fixrow = cpool.tile([1, D], BF16, tag="fixrow")

            wproj = [wppool.tile([64, D], BF16, tag=f"wp{h}", name=f"wp{h}") for h in range(H)]
            for h in range(H):
                nc.sync.dma_start(out=wproj[h][:, :], in_=wproj_d[64 * h:64 * (h + 1), :])

            ktil = [qkvpool.tile([128, KP], BF16, tag=f"kt{i}", name=f"kt{i}") for i in range(8)]
            qtil = [qkvpool.tile([128, QC], BF16, tag=f"qt{i}", name=f"qt{i}") for i in range(8)]
            vaug = [qkvpool.tile([128, VW], BF16, tag=f"va{s}", name=f"va{s}") for s in range(NST)]

            with tc.tile_pool(name="xp", bufs=1) as xpool:
                xT = [xpool.tile([128, N], BF16, tag=f"xT{i}", name=f"xT{i}") for i in range(8)]
                xTq = [xpool.tile([128, QC], BF16, tag=f"xTq{i}", name=f"xTq{i}") for i in range(8)]
                for i in range(8):
                    nc.sync.dma_start(out=xT[i][:, :], in_=xT_d[128 * i:128 * (i + 1), :])
                    nc.sync.dma_start(out=xTq[i][:, :], in_=xTq_d[128 * i:128 * (i + 1), :])

                # ---------------- phase 1a: K^T, Q^T projections ----------
                with tc.tile_pool(name="wqk", bufs=1) as wqkpool:
                    wqk = [wqkpool.tile([128, 2 * D], BF16, tag=f"wq{i}", name=f"wq{i}") for i in range(8)]
                    for i in range(8):
                        nc.sync.dma_start(out=wqk[i][:, :], in_=wqkv_d[128 * i:128 * (i + 1), 0:2 * D])
                    for ft in range(8):
                        for (coff, cw) in kchunks:
                            ps = psA.tile([128, 512], F32, tag="psA")
                            for xk in range(8):
                                nc.tensor.matmul(ps[:, 0:cw],
                                                 wqk[xk][:, D + 128 * ft:D + 128 * (ft + 1)],
                                                 xT[xk][:, coff:coff + cw],
                                                 start=(xk == 0), stop=(xk == 7))
                            nc.scalar.activation(ktil[ft][:, coff:coff + cw], ps[:, 0:cw],
                                                 AF.Identity, bias=biaspp[:, 8 + ft:9 + ft])
                    for ft in range(8):
                        ps = psA.tile([128, 512], F32, tag="psA")
                        for xk in range(8):
                            nc.tensor.matmul(ps[:, :],
                                             wqk[xk][:, 128 * ft:128 * (ft + 1)],
                                             xTq[xk][:, :],
                                             start=(xk == 0), stop=(xk == 7))
                        nc.scalar.activation(qtil[ft][:, :], ps[:, :], AF.Identity,
                                             bias=biaspp[:, ft:ft + 1], scale=1.0 / 8.0)

                # ---------------- phase 1b: V (augmented) + mean ----------
                with tc.tile_pool(name="wv", bufs=1) as wvpool:
                    wv = [wvpool.tile([128, D], BF16, tag=f"wv{i}", name=f"wv{i}") for i in range(8)]
                    for i in range(8):
                        nc.sync.dma_start(out=wv[i][:, :], in_=wqkv_d[128 * i:128 * (i + 1), 2 * D:3 * D])
                    for st in range(NST):
                        nc.vector.memset(vaug[st][:, :], 0.0)
                        diag = vaug[st][:, 64:64 + 82 * 16].rearrange("p (h c) -> p h c", c=82)[:, :, 0:1]
                        nc.vector.memset(diag, 1.0)
                        for ch in range(2):
                            ps = psA.tile([128, 512], F32, tag="psA")
                            for xk in range(8):
                                nc.tensor.matmul(ps[:, :],
                                                 xT[xk][:, 128 * st:128 * (st + 1)],
                                                 wv[xk][:, 512 * ch:512 * (ch + 1)],
                                                 start=(xk == 0), stop=False)
                            nc.tensor.matmul(ps[:, :], ones1[:, :],
                                             bvrow[:, 512 * ch:512 * (ch + 1)],
                                             start=False, stop=True)
                            dst = vaug[st][:, 81 * 8 * ch:81 * 8 * ch + 81 * 8] \
                                .rearrange("p (h c) -> p h c", c=81)[:, :, 0:64]
                            nc.vector.tensor_copy(out=dst, in_=ps[:, :])

                    # mean(V) chain
                    xsum = cpool.tile([128, 8], F32, tag="xsum")
                    xsum_bf = cpool.tile([128, 8], BF16, tag="xsum_bf")
                    for i in range(8):
                        nc.vector.reduce_sum(xsum[:, i:i + 1], xT[i][:, :],
                                             axis=mybir.AxisListType.X)
                    nc.vector.tensor_copy(out=xsum_bf[:, :], in_=xsum[:, :])
                    for h in range(H):
                        ps = psS.tile([128, 512], F32, tag="psS")
                        for xk in range(8):
                            nc.tensor.matmul(ps[0:64, 0:1],
                                             wv[xk][:, 64 * h:64 * (h + 1)],
                                             xsum_bf[:, xk:xk + 1],
                                             start=(xk == 0), stop=(xk == 7))
                        nc.scalar.activation(meanVT[:, h:h + 1], ps[0:64, 0:1], AF.Identity,
                                             bias=bvH[:, h:h + 1], scale=1.0 / N)
                    for ch in range(2):
                        ps = psS.tile([128, 512], F32, tag="psS")
                        for h in range(H):
                            nc.tensor.matmul(ps[0:1, :], meanVT[:, h:h + 1],
                                             wproj[h][:, 512 * ch:512 * (ch + 1)],
                                             start=(h == 0), stop=(h == 15))
                        nc.vector.tensor_copy(out=fixrow[:, 512 * ch:512 * (ch + 1)],
                                              in_=ps[0:1, :])

            # ---------------- phase 3: attention ----------------
            with tc.tile_pool(name="attn", bufs=1) as apool, \
                 tc.tile_pool(name="ppool", bufs=15) as ppool:
                qiota = apool.tile([128, QC], F32, tag="qiota")
                nc.sync.dma_start(out=qiota[:, :], in_=qiota_d[:, :])
                qm = apool.tile([128, QC], F32, tag="qm")
                nc.vector.tensor_scalar(out=qm[:, :], in0=qiota[:, :],
                                        scalar1=v128[:, 0:1], scalar2=None,
                                        op0=OP.is_lt)
                iqrow = apool.tile([1, QC], BF16, tag="iqrow")
                nc.vector.tensor_scalar(out=iqrow[:, :], in0=qiota[0:1, :],
                                        scalar1=v128[0:1, 0:1], scalar2=None,
                                        op0=OP.is_ge)
                onesq = apool.tile([1, QC], BF16, tag="onesq")
                nc.vector.memset(onesq[:, :], 1.0)
                bprow = apool.tile([1, D], BF16, tag="bprow")
                nc.sync.dma_start(out=bprow[:, :], in_=brows_d[0:1, D:2 * D])
                esel = apool.tile([128, H * 64], BF16, tag="esel")
                nc.sync.dma_start(out=esel[64:80, :], in_=esel_d[:, :])

                dacc = apool.tile([128, QC], F32, tag="dacc")
                nc.vector.memset(dacc[64:80, :], 0.0)
                utun = [apool.tile([64, QC], BF16, tag=f"uu{h}", name=f"uu{h}") for h in range(H)]
                for h in range(H):
                    ft, hb = h // 2, 64 * (h % 2)
                    ptil = []
                    for kt in range(KT):
                        ps = psA.tile([128, 512], F32, tag="psA")
                        nc.tensor.matmul(ps[:, :],
                                         ktil[ft][hb:hb + 64, 128 * kt:128 * (kt + 1)],
                                         qtil[ft][hb:hb + 64, :],
                                         start=True, stop=True)
                        pt = ppool.tile([128, QC], BF16, tag="pt")
                        nc.scalar.activation(pt[:, :], ps[:, :], AF.Exp,
                                             bias=kb[:, kt:kt + 1])
                        ptil.append(pt)
                    pv = psPV.tile([80, QC], F32, tag="pv")
                    for kt in range(KT):
                        nc.tensor.matmul(pv[:, :],
                                         vaug[kt][:, 81 * h:81 * h + 80],
                                         ptil[kt][:, :],
                                         start=(kt == 0), stop=(kt == KT - 1))
                    nc.vector.tensor_copy(out=utun[h][:, :], in_=pv[0:64, :])
                    nc.vector.tensor_tensor(out=dacc[64:80, :], in0=dacc[64:80, :],
                                            in1=pv[64:80, :], op=OP.add)

                rr = apool.tile([128, QC], F32, tag="rr")
                rmk = apool.tile([128, QC], BF16, tag="rmk")
                nc.vector.tensor_scalar(out=dacc[64:80, :], in0=dacc[64:80, :],
                                        scalar1=1e-30, scalar2=None, op0=OP.max)
                nc.vector.reciprocal(out=rr[64:80, :], in_=dacc[64:80, :])
                nc.vector.tensor_tensor(out=rmk[64:80, :], in0=rr[64:80, :],
                                        in1=qm[64:80, :], op=OP.mult)

                ut = [apool.tile([64, QC], BF16, tag=f"ut{h}", name=f"ut{h}") for h in range(H)]
                for h in range(H):
                    rb = psR.tile([64, QC], F32, tag="rb")
                    nc.tensor.matmul(rb[:, :], esel[64:80, 64 * h:64 * h + 64],
                                     rmk[64:80, :], start=True, stop=True)
                    nc.vector.tensor_tensor(out=ut[h][:, :], in0=utun[h][:, :],
                                            in1=rb[:, :], op=OP.mult)

                # ---------------- phase 4: output projection ----------------
                for mt in range(4):
                    outsb = apool.tile([128, D], F32, tag="outsb", bufs=2)
                    for ch in range(2):
                        ps = psA.tile([128, 512], F32, tag="psA")
                        for h in range(H):
                            nc.tensor.matmul(ps[:, :],
                                             ut[h][:, 128 * mt:128 * (mt + 1)],
                                             wproj[h][:, 512 * ch:512 * (ch + 1)],
                                             start=(h == 0), stop=False)
                        nc.tensor.matmul(ps[:, :], onesq[:, 128 * mt:128 * (mt + 1)],
                                         bprow[:, 512 * ch:512 * (ch + 1)],
                                         start=False, stop=False)
                        nc.tensor.matmul(ps[:, :], iqrow[:, 128 * mt:128 * (mt + 1)],
                                         fixrow[:, 512 * ch:512 * (ch + 1)],
                                         start=False, stop=True)
                        nc.vector.tensor_copy(out=outsb[:, 512 * ch:512 * (ch + 1)],
                                              in_=ps[:, :])
                    nc.sync.dma_start(out=out_d[128 * mt:128 * (mt + 1), :],
                                      in_=outsb[:, :])
    nc.compile()
    return nc


def _prep(x, vaild_num, W_qkv, b_qkv, W_proj, b_proj):
    v = np.asarray(vaild_num).astype(np.int64)
    vmax = int(max(1, v.max()))
    KT = (vmax + 127) // 128
    wqkv_bf = np.ascontiguousarray(W_qkv.astype(BF))
    wproj_bf = np.ascontiguousarray(W_proj.astype(BF))
    biaspp = np.empty((128, 16), np.float32)
    biaspp[:, 0:8] = (b_qkv[0:D].reshape(8, 128).T) / 8.0
    biaspp[:, 8:16] = b_qkv[D:2 * D].reshape(8, 128).T
    bvH = np.ascontiguousarray(b_qkv[2 * D:3 * D].reshape(16, 64).T.astype(np.float32))
    brows = np.zeros((1, 2 * D), BF)
    brows[0, 0:D] = b_qkv[2 * D:3 * D].astype(BF)
    brows[0, D:2 * D] = b_proj.astype(BF)
    kiota = (np.arange(128, dtype=np.float32)[:, None]
             + 128.0 * np.arange(KT, dtype=np.float32)[None, :])
    esel_np = np.zeros((H, H * 64), BF)
    for h in range(H):
        esel_np[h, 64 * h:64 * (h + 1)] = 1.0
    in_maps = []
    for c in range(NCORES):
        b, j = c // 4, c % 4
        q0 = QC * j
        xTb = np.ascontiguousarray(x[b].T.astype(BF))
        m = {
            "xT": xTb,
            "xTq": np.ascontiguousarray(xTb[:, q0:q0 + QC]),
            "wqkv": wqkv_bf,
            "wproj": wproj_bf,
            "biaspp": biaspp,
            "bvH": bvH,
            "brows": brows,
            "qiota": np.broadcast_to(
                (q0 + np.arange(QC, dtype=np.float32))[None, :], (128, QC)).copy(),
            "v128": np.full((128, 1), float(v[b]), np.float32),
            "kiota": kiota,
            "esel": esel_np,
        }
        in_maps.append(m)
    return KT, in_maps


def _install_ntff_hook():
    """Provide antenv.axon_hooks backed by trn_boot's ctypes NTFF profiler."""
    import sys, types
    try:
        from antenv import axon_hooks  # noqa: F401
        return
    except ImportError:
        pass
    mod = types.ModuleType("antenv.axon_hooks")
    _h = [None]
    mod.set_axon_ntff_profile_hook = lambda h: _h.__setitem__(0, h)
    mod.get_axon_ntff_profile_hook = lambda: _h[0]
    sys.modules["antenv.axon_hooks"] = mod
    try:
        from trn_agent_boot.trn_boot import _ntff_profile_via_ctypes
        hook = _ntff_profile_via_ctypes("/opt/axon/libaxon_pjrt.so")
        mod.set_axon_ntff_profile_hook(hook)
    except Exception as e:  # profiling degrades, run still works
        print("ntff hook install failed:", e)


_CACHE = {}


def kernel(x, vaild_num, W_qkv, b_qkv, W_proj, b_proj, _trace=False):
    x = np.asarray(x, np.float32)
    KT, in_maps = _prep(np.asarray(x, np.float32), vaild_num,
                        np.asarray(W_qkv, np.float32), np.asarray(b_qkv, np.float32),
                        np.asarray(W_proj, np.float32), np.asarray(b_proj, np.float32))
    _install_ntff_hook()
    if KT not in _CACHE:
        _CACHE[KT] = build_nc(KT)
    nc = _CACHE[KT]
    res = run_bass_kernel_spmd(nc, in_maps, core_ids=list(range(NCORES)),
                               trace=_trace)
    out = np.empty((B, N, D), np.float32)
    for c in range(NCORES):
        b, j = c // 4, c % 4
        out[b, QC * j:QC * (j + 1), :] = res.results[c]["out"]
    kernel._last_exec_ns = res.exec_time_ns
    return out


# revision 10
# speedup vs baseline: 1.3409x; 1.3409x over previous
"""Trainium2 Bass kernel for nn_Attention (B=2, N=2048, H=16, hd=64, D=1024).

Strategy (8 NeuronCores, no collectives):
  core c -> batch b=c//4, query chunk j=c%4 (512 rows). Each core computes
  K,V for its batch over the first KP=KT*128 key rows (KT specialized to the
  runtime vaild_num; masked tail keys contribute exp(-1e9)=0), Q for its own
  512 rows, attention in transposed layout (scores S^T[k,q] so the key-validity
  mask is a per-partition bias folded into the Exp activation), and the output
  projection. Per-sample valid-length semantics:
    - keys k >= v: masked via bias -1e9 before exp -> P=0
    - queries q >= v: reference gives uniform softmax over ALL 2048 keys ->
      out row = mean(V_full) @ W_proj + b_proj. Implemented by zeroing the
      normalizer for invalid q and adding a rank-1 fixup row in the proj
      matmul: out += (q>=v) * (mean(V) @ W_proj).
  Softmax denominators ride along the PV matmul as 16 extra stationary
  columns (diagonal ones), landing head h's denominator on PSUM partition
  64+h so all 16 can be batched for one reciprocal.

Compute dtype bf16 (fp32 PSUM accumulation); fp32 in/out.
"""

import numpy as np
import ml_dtypes

import concourse.mybir as mybir
import concourse.tile as tile
from concourse import bacc
from concourse.bass_utils import run_bass_kernel_spmd

F32 = mybir.dt.float32
BF16 = mybir.dt.bfloat16
AF = mybir.ActivationFunctionType
OP = mybir.AluOpType

H, HD, D, N, B, NCORES = 16, 64, 1024, 2048, 2, 8
QC = 512            # query rows per core
NEG = -1e9
BF = ml_dtypes.bfloat16


def build_nc(KT, BT0):
    """SPMD graph for KT key tiles; tiles < BT0 need no key mask (vmin-safe)."""
    NPAIR = BT0 // 2
    KP = KT * 128
    # free-dim chunks for the K^T projection output (seq axis)
    kchunks = []
    off = 0
    while off < KP:
        w = min(512, KP - off)
        kchunks.append((off, w))
        off += w
    NST = KT          # number of 128-row seq tiles for V
    VW = 64 + 82 * 16  # vaug width: blocks stride 81, diag col at 82h+64

    nc = bacc.Bacc(None, target_bir_lowering=False)

    # ---------------- DRAM parameters (per-core host-prepared shards) -------
    xT_d = nc.declare_dram_parameter("xT", [D, N], BF16, isOutput=False)
    xTq_d = nc.declare_dram_parameter("xTq", [D, QC], BF16, isOutput=False)
    wqkv_d = nc.declare_dram_parameter("wqkv", [D, 3 * D], BF16, isOutput=False)
    wproj_d = nc.declare_dram_parameter("wproj", [D, D], BF16, isOutput=False)
    # per-partition biases: [128, 8] each; col ft covers features ft*128..+128
    # bqT pre-scaled by 1/sqrt(hd)
    biaspp_d = nc.declare_dram_parameter("biaspp", [128, 16], F32, isOutput=False)
    bvH_d = nc.declare_dram_parameter("bvH", [64, 16], F32, isOutput=False)  # V bias per head
    brows_d = nc.declare_dram_parameter("brows", [1, 2 * D], BF16, isOutput=False)  # [bv | bproj]
    qiota_d = nc.declare_dram_parameter("qiota", [128, QC], F32, isOutput=False)
    v128_d = nc.declare_dram_parameter("v128", [128, 1], F32, isOutput=False)
    kiota_d = nc.declare_dram_parameter("kiota", [128, KT], F32, isOutput=False)
    esel_d = nc.declare_dram_parameter("esel", [H, H * 64], BF16, isOutput=False)
    out_d = nc.declare_dram_parameter("out", [QC, D], F32, isOutput=True)

    with tile.TileContext(nc) as tc:
        with tc.tile_pool(name="const", bufs=1) as cpool, \
             tc.tile_pool(name="qkv", bufs=1) as qkvpool, \
             tc.tile_pool(name="wpp", bufs=1) as wppool, \
             tc.tile_pool(name="psA", bufs=2, space="PSUM") as psA, \
             tc.tile_pool(name="psBig", bufs=2, space="PSUM") as psBig, \
             tc.tile_pool(name="psPV", bufs=2, space="PSUM") as psPV:

            # ---------------- constants / small inputs ----------------
            biaspp = cpool.tile([128, 16], F32, tag="biaspp")
            bvH = cpool.tile([64, 16], F32, tag="bvH")
            v128 = cpool.tile([128, 1], F32, tag="v128")
            kiota = cpool.tile([128, KT], F32, tag="kiota")
            nc.sync.dma_start(out=biaspp[:, :], in_=biaspp_d[:, :])
            nc.sync.dma_start(out=bvH[:, :], in_=bvH_d[:, :])
            nc.sync.dma_start(out=v128[:, :], in_=v128_d[:, :])
            nc.sync.dma_start(out=kiota[:, :], in_=kiota_d[:, :])
            bvrow = cpool.tile([1, D], BF16, tag="bvrow")
            nc.sync.dma_start(out=bvrow[:, :], in_=brows_d[0:1, 0:D])
            ones1 = cpool.tile([1, 128], BF16, tag="ones1")
            nc.vector.memset(ones1[:, :], 1.0)
            kb = cpool.tile([128, KT], F32, tag="kb")
            nc.vector.tensor_scalar(out=kb[:, :], in0=kiota[:, :],
                                    scalar1=v128[:, 0:1], scalar2=NEG,
                                    op0=OP.is_ge, op1=OP.mult)
            meanVT = cpool.tile([64, H], BF16, tag="meanVT")
            fixrow = cpool.tile([1, D], BF16, tag="fixrow")

            wproj = [wppool.tile([64, D], BF16, tag=f"wp{h}", name=f"wp{h}") for h in range(H)]

            ktil = [qkvpool.tile([128, KP], BF16, tag=f"kt{i}", name=f"kt{i}") for i in range(8)]
            qtil = [qkvpool.tile([128, QC], BF16, tag=f"qt{i}", name=f"qt{i}") for i in range(8)]
            vaug = [qkvpool.tile([128, VW], BF16, tag=f"va{s}", name=f"va{s}") for s in range(NST)]

            with tc.tile_pool(name="xp", bufs=1) as xpool:
                xT = [xpool.tile([128, N], BF16, tag=f"xT{i}", name=f"xT{i}") for i in range(8)]
                xTq = [xpool.tile([128, QC], BF16, tag=f"xTq{i}", name=f"xTq{i}") for i in range(8)]

                # ---------------- phase 1a: K^T, Q^T projections ----------
                with tc.tile_pool(name="wqk", bufs=1) as wqkpool:
                    wqk = [wqkpool.tile([128, 2 * D], BF16, tag=f"wq{i}", name=f"wq{i}") for i in range(8)]
                    for i in range(8):
                        nc.sync.dma_start(out=xT[i][:, :], in_=xT_d[128 * i:128 * (i + 1), :])
                        nc.sync.dma_start(out=wqk[i][:, :], in_=wqkv_d[128 * i:128 * (i + 1), 0:2 * D])
                    for i in range(8):
                        nc.sync.dma_start(out=xTq[i][:, :], in_=xTq_d[128 * i:128 * (i + 1), :])
                    for ft in range(8):
                        for (coff, cw) in kchunks:
                            ps = psA.tile([128, 512], F32, tag="psA")
                            for xk in range(8):
                                nc.tensor.matmul(ps[:, 0:cw],
                                                 wqk[xk][:, D + 128 * ft:D + 128 * (ft + 1)],
                                                 xT[xk][:, coff:coff + cw],
                                                 start=(xk == 0), stop=(xk == 7))
                            nc.scalar.activation(ktil[ft][:, coff:coff + cw], ps[:, 0:cw],
                                                 AF.Identity, bias=biaspp[:, 8 + ft:9 + ft])
                    for ft in range(8):
                        ps = psA.tile([128, 512], F32, tag="psA")
                        for xk in range(8):
                            nc.tensor.matmul(ps[:, :],
                                             wqk[xk][:, 128 * ft:128 * (ft + 1)],
                                             xTq[xk][:, :],
                                             start=(xk == 0), stop=(xk == 7))
                        nc.scalar.activation(qtil[ft][:, :], ps[:, :], AF.Identity,
                                             bias=biaspp[:, ft:ft + 1], scale=1.0 / 8.0)

                # ---------------- phase 1b: V (augmented) + mean ----------
                with tc.tile_pool(name="wv", bufs=1) as wvpool:
                    wv = [wvpool.tile([128, D], BF16, tag=f"wv{i}", name=f"wv{i}") for i in range(8)]
                    for i in range(8):
                        nc.sync.dma_start(out=wv[i][:, :], in_=wqkv_d[128 * i:128 * (i + 1), 2 * D:3 * D])
                    for st in range(NST):
                        nc.vector.memset(vaug[st][:, :], 0.0)
                        diag = vaug[st][:, 64:64 + 82 * 16].rearrange("p (h c) -> p h c", c=82)[:, :, 0:1]
                        nc.vector.memset(diag, 1.0)
                        for ch in range(2):
                            ps = psA.tile([128, 512], F32, tag="psA")
                            for xk in range(8):
                                nc.tensor.matmul(ps[:, :],
                                                 xT[xk][:, 128 * st:128 * (st + 1)],
                                                 wv[xk][:, 512 * ch:512 * (ch + 1)],
                                                 start=(xk == 0), stop=False)
                            nc.tensor.matmul(ps[:, :], ones1[:, :],
                                             bvrow[:, 512 * ch:512 * (ch + 1)],
                                             start=False, stop=True)
                            dst = vaug[st][:, 81 * 8 * ch:81 * 8 * ch + 81 * 8] \
                                .rearrange("p (h c) -> p h c", c=81)[:, :, 0:64]
                            nc.vector.tensor_copy(out=dst, in_=ps[:, :])

                    # mean(V) chain
                    for h in range(H):
                        nc.sync.dma_start(out=wproj[h][:, :], in_=wproj_d[64 * h:64 * (h + 1), :])
                    xsum = cpool.tile([128, 8], F32, tag="xsum")
                    xsum_bf = cpool.tile([128, 8], BF16, tag="xsum_bf")
                    for i in range(8):
                        nc.vector.reduce_sum(xsum[:, i:i + 1], xT[i][:, :],
                                             axis=mybir.AxisListType.X)
                    nc.vector.tensor_copy(out=xsum_bf[:, :], in_=xsum[:, :])
                    for h in range(H):
                        ps = psA.tile([128, 512], F32, tag="psA")
                        for xk in range(8):
                            nc.tensor.matmul(ps[0:64, 0:1],
                                             wv[xk][:, 64 * h:64 * (h + 1)],
                                             xsum_bf[:, xk:xk + 1],
                                             start=(xk == 0), stop=(xk == 7))
                        nc.scalar.activation(meanVT[:, h:h + 1], ps[0:64, 0:1], AF.Identity,
                                             bias=bvH[:, h:h + 1], scale=1.0 / N)
                    for ch in range(2):
                        ps = psA.tile([128, 512], F32, tag="psA")
                        for h in range(H):
                            nc.tensor.matmul(ps[0:1, :], meanVT[:, h:h + 1],
                                             wproj[h][:, 512 * ch:512 * (ch + 1)],
                                             start=(h == 0), stop=(h == 15))
                        nc.vector.tensor_copy(out=fixrow[:, 512 * ch:512 * (ch + 1)],
                                              in_=ps[0:1, :])

            # ---------------- phase 3: attention ----------------
            with tc.tile_pool(name="attn", bufs=1) as apool, \
                 tc.tile_pool(name="ppool", bufs=2) as ppool:
                qiota = apool.tile([128, QC], F32, tag="qiota")
                nc.sync.dma_start(out=qiota[:, :], in_=qiota_d[:, :])
                qm = apool.tile([128, QC], F32, tag="qm")
                nc.vector.tensor_scalar(out=qm[:, :], in0=qiota[:, :],
                                        scalar1=v128[:, 0:1], scalar2=None,
                                        op0=OP.is_lt)
                iqrow = apool.tile([1, QC], BF16, tag="iqrow")
                nc.vector.tensor_scalar(out=iqrow[:, :], in0=qiota[0:1, :],
                                        scalar1=v128[0:1, 0:1], scalar2=None,
                                        op0=OP.is_ge)
                onesq = apool.tile([1, QC], BF16, tag="onesq")
                nc.vector.memset(onesq[:, :], 1.0)
                bprow = apool.tile([1, D], BF16, tag="bprow")
                nc.sync.dma_start(out=bprow[:, :], in_=brows_d[0:1, D:2 * D])
                esel = apool.tile([128, H * 64], BF16, tag="esel")
                nc.sync.dma_start(out=esel[64:80, :], in_=esel_d[:, :])

                dacc = apool.tile([128, QC], F32, tag="dacc")
                nc.vector.memset(dacc[64:80, :], 0.0)
                utun = [apool.tile([64, QC], BF16, tag=f"uu{h}", name=f"uu{h}") for h in range(H)]
                for h in range(H):
                    ft, hb = h // 2, 64 * (h % 2)
                    ptil = {}
                    for pi in range(NPAIR):
                        ps = psBig.tile([128, 2 * QC], F32, tag="psBig")
                        for s in range(2):
                            kt = 2 * pi + s
                            nc.tensor.matmul(ps[:, QC * s:QC * (s + 1)],
                                             ktil[ft][hb:hb + 64, 128 * kt:128 * (kt + 1)],
                                             qtil[ft][hb:hb + 64, :],
                                             start=True, stop=True)
                        pt = ppool.tile([128, 2 * QC], BF16, tag="ptp", bufs=8)
                        nc.scalar.activation(pt[:, :], ps[:, :], AF.Exp)
                        ptil[2 * pi] = pt[:, 0:QC]
                        ptil[2 * pi + 1] = pt[:, QC:2 * QC]
                    for kt in range(2 * NPAIR, KT):
                        ps = psA.tile([128, 512], F32, tag="psA")
                        nc.tensor.matmul(ps[:, :],
                                         ktil[ft][hb:hb + 64, 128 * kt:128 * (kt + 1)],
                                         qtil[ft][hb:hb + 64, :],
                                         start=True, stop=True)
                        pt = ppool.tile([128, QC], BF16, tag="pts", bufs=8)
                        nc.scalar.activation(pt[:, :], ps[:, :], AF.Exp,
                                             bias=kb[:, kt:kt + 1])
                        ptil[kt] = pt[:, :]
                    pv = psPV.tile([80, QC], F32, tag="pv")
                    for kt in range(KT):
                        nc.tensor.matmul(pv[:, :],
                                         vaug[kt][:, 81 * h:81 * h + 80],
                                         ptil[kt],
                                         start=(kt == 0), stop=(kt == KT - 1))
                    nc.vector.tensor_copy(out=utun[h][:, :], in_=pv[0:64, :])
                    nc.vector.tensor_tensor(out=dacc[64:80, :], in0=dacc[64:80, :],
                                            in1=pv[64:80, :], op=OP.add)

                rr = apool.tile([128, QC], F32, tag="rr")
                rmk = apool.tile([128, QC], BF16, tag="rmk")
                nc.vector.tensor_scalar(out=dacc[64:80, :], in0=dacc[64:80, :],
                                        scalar1=1e-30, scalar2=None, op0=OP.max)
                nc.vector.reciprocal(out=rr[64:80, :], in_=dacc[64:80, :])
                nc.vector.tensor_tensor(out=rmk[64:80, :], in0=rr[64:80, :],
                                        in1=qm[64:80, :], op=OP.mult)

                ut = [apool.tile([64, QC], BF16, tag=f"ut{h}", name=f"ut{h}") for h in range(H)]
                for h in range(H):
                    rb = psA.tile([64, QC], F32, tag="psA")
                    nc.tensor.matmul(rb[:, :], esel[64:80, 64 * h:64 * h + 64],
                                     rmk[64:80, :], start=True, stop=True)
                    nc.vector.tensor_tensor(out=ut[h][:, :], in0=utun[h][:, :],
                                            in1=rb[:, :], op=OP.mult)

                # ---------------- phase 4: output projection ----------------
                for mt in range(4):
                    outsb = apool.tile([128, D], F32, tag="outsb", bufs=2)
                    for ch in range(2):
                        ps = psA.tile([128, 512], F32, tag="psA")
                        for h in range(H):
                            nc.tensor.matmul(ps[:, :],
                                             ut[h][:, 128 * mt:128 * (mt + 1)],
                                             wproj[h][:, 512 * ch:512 * (ch + 1)],
                                             start=(h == 0), stop=False)
                        nc.tensor.matmul(ps[:, :], onesq[:, 128 * mt:128 * (mt + 1)],
                                         bprow[:, 512 * ch:512 * (ch + 1)],
                                         start=False, stop=False)
                        nc.tensor.matmul(ps[:, :], iqrow[:, 128 * mt:128 * (mt + 1)],
                                         fixrow[:, 512 * ch:512 * (ch + 1)],
                                         start=False, stop=True)
                        nc.vector.tensor_copy(out=outsb[:, 512 * ch:512 * (ch + 1)],
                                              in_=ps[:, :])
                    nc.sync.dma_start(out=out_d[128 * mt:128 * (mt + 1), :],
                                      in_=outsb[:, :])
    nc.compile()
    return nc


def _prep(x, vaild_num, W_qkv, b_qkv, W_proj, b_proj):
    v = np.asarray(vaild_num).astype(np.int64)
    vmax = int(max(1, v.max()))
    KT = (vmax + 127) // 128
    BT0 = min(int(v.min()) // 128, KT)
    wqkv_bf = np.ascontiguousarray(W_qkv.astype(BF))
    wproj_bf = np.ascontiguousarray(W_proj.astype(BF))
    biaspp = np.empty((128, 16), np.float32)
    biaspp[:, 0:8] = (b_qkv[0:D].reshape(8, 128).T) / 8.0
    biaspp[:, 8:16] = b_qkv[D:2 * D].reshape(8, 128).T
    bvH = np.ascontiguousarray(b_qkv[2 * D:3 * D].reshape(16, 64).T.astype(np.float32))
    brows = np.zeros((1, 2 * D), BF)
    brows[0, 0:D] = b_qkv[2 * D:3 * D].astype(BF)
    brows[0, D:2 * D] = b_proj.astype(BF)
    kiota = (np.arange(128, dtype=np.float32)[:, None]
             + 128.0 * np.arange(KT, dtype=np.float32)[None, :])
    esel_np = np.zeros((H, H * 64), BF)
    for h in range(H):
        esel_np[h, 64 * h:64 * (h + 1)] = 1.0
    in_maps = []
    for c in range(NCORES):
        b, j = c // 4, c % 4
        q0 = QC * j
        xTb = np.ascontiguousarray(x[b].T.astype(BF))
        m = {
            "xT": xTb,
            "xTq": np.ascontiguousarray(xTb[:, q0:q0 + QC]),
            "wqkv": wqkv_bf,
            "wproj": wproj_bf,
            "biaspp": biaspp,
            "bvH": bvH,
            "brows": brows,
            "qiota": np.broadcast_to(
                (q0 + np.arange(QC, dtype=np.float32))[None, :], (128, QC)).copy(),
            "v128": np.full((128, 1), float(v[b]), np.float32),
            "kiota": kiota,
            "esel": esel_np,
        }
        in_maps.append(m)
    return KT, BT0, in_maps


def _install_ntff_hook():
    """Provide antenv.axon_hooks backed by trn_boot's ctypes NTFF profiler."""
    import sys, types
    try:
        from antenv import axon_hooks  # noqa: F401
        return
    except ImportError:
        pass
    mod = types.ModuleType("antenv.axon_hooks")
    _h = [None]
    mod.set_axon_ntff_profile_hook = lambda h: _h.__setitem__(0, h)
    mod.get_axon_ntff_profile_hook = lambda: _h[0]
    sys.modules["antenv.axon_hooks"] = mod
    try:
        from trn_agent_boot.trn_boot import _ntff_profile_via_ctypes
        hook = _ntff_profile_via_ctypes("/opt/axon/libaxon_pjrt.so")
        mod.set_axon_ntff_profile_hook(hook)
    except Exception as e:  # profiling degrades, run still works
        print("ntff hook install failed:", e)


_CACHE = {}


def kernel(x, vaild_num, W_qkv, b_qkv, W_proj, b_proj, _trace=False):
    x = np.asarray(x, np.float32)
    KT, BT0, in_maps = _prep(np.asarray(x, np.float32), vaild_num,
                             np.asarray(W_qkv, np.float32), np.asarray(b_qkv, np.float32),
                             np.asarray(W_proj, np.float32), np.asarray(b_proj, np.float32))
    _install_ntff_hook()
    if (KT, BT0) not in _CACHE:
        _CACHE[(KT, BT0)] = build_nc(KT, BT0)
    nc = _CACHE[(KT, BT0)]
    res = run_bass_kernel_spmd(nc, in_maps, core_ids=list(range(NCORES)),
                               trace=_trace)
    out = np.empty((B, N, D), np.float32)
    for c in range(NCORES):
        b, j = c // 4, c % 4
        out[b, QC * j:QC * (j + 1), :] = res.results[c]["out"]
    kernel._last_exec_ns = res.exec_time_ns
    return out
